# revision 48
# baseline (speedup 1.0000x reference)
"""Trainium2 Bass kernel for nn_CryptoNet: 3-layer LSTM + per-step BatchNorm + 2-layer head.

Strategy: 8-way data parallel over batch (128 samples/core), zero cross-core
communication (BN uses per-shard batch stats; measured rel err vs full-batch
stats ~1.8e-3).

v5 changes over v4 (TimelineSim 1.836ms -> 1.558ms; rel err unchanged at
6.03e-3).  The superstep was latency-bound (all engines <60% busy), so the
work targets the L1 recurrence cycle and the in-order engine-queue coupling:
  - BN stats via per-chunk bn_stats/bn_aggr on the SBUF h.T copies (replaces
    8 scalar_tensor_tensor passes); the rsqrt Newton chain moved entirely to
    DVE where [128,5] ops cost ~65ns and feed the DVE z-applies directly.
  - L2 pipeline split across supersteps: gates+sigmoid+cell update at s, the
    tanh/h/transpose/copy/stats tail at the FRONT of s+1, so the next
    superstep's L1 sigmoid never head-blocks behind L2's late activations.
    L3 restructured the same way (tail one superstep later).
  - Separate PSUM tiles for L1 vs L3 gates (and separate ACT ops/tiles per
    layer): next-superstep L1 matmuls no longer wait for L3's late PSUM
    readers; tanh(c1) no longer waits on Pool's c3 cell math.
  - L1 chain ACT split: sigmoid(i,f) first (unblocks cell math), then
    sigmoid(o) / tanh(g) while DVE runs.
  - head: first-layer matmul issued at superstep top (z3T ready), relu on
    DVE, z2/z3 BN-applies on Pool (Pool cannot touch PSUM; all its operands
    are SBUF).
  - per-step sigmoid/1-p of the head amortized into one strip op at the end
    (unchanged from v4); cell state c in bf16 (unchanged from v4).
"""

import sys
import numpy as np

for p in ("/opt/trn_rl_repo", "/opt/trn_rl_repo/concourse"):
    if p not in sys.path:
        sys.path.insert(0, p)

B, T, I = 1024, 256, 128
T_STEPS = T  # override for small-scale testing
T_RUN = None  # loop steps; defaults to T_STEPS
H1, H2, H3 = 256, 256, 32
NCORES = 8
BL = B // NCORES  # local batch per core = 128
EPS = 1e-5

_CACHE = {}


def _gate_perm(H):
    # torch gate order (i, f, g, o) -> (i, f, o, g)
    idx = np.arange(4 * H)
    i, f, g, o = np.split(idx, 4)
    return np.concatenate([i, f, o, g])


def _build(dt_w, dt_x, run=None):
    import concourse.bass as bass
    import concourse.tile as tile
    import concourse.mybir as mybir
    from concourse import bacc
    from concourse.masks import make_identity

    f32 = mybir.dt.float32
    u32 = mybir.dt.uint32
    f32r = mybir.dt.float32r
    AF = mybir.ActivationFunctionType
    OP = mybir.AluOpType
    bf16 = mybir.dt.bfloat16

    nc = bacc.Bacc("TRN2", target_bir_lowering=False, debug=False,
                   num_devices=NCORES)

    # gL1 PSUM layout (f32 cols, 2 banks): if [0:512), o [512:768),
    # g [768:1024).  L3 gates live in their own 1-bank tile: ifo [0:96),
    # g [96:128) -- separate tiles so next-superstep L1 matmuls never wait
    # on L3's late ACT readers.
    L1_IF = 0        # 512 wide
    L1_O = 512       # 256 wide
    L1_G = 768       # 256 wide
    L3_IFO = 0       # 96 wide (in gL3)
    L3_G = 96        # 32 wide (in gL3)

    with tile.TileContext(nc) as tc:
        dr = lambda name, shape, dt: nc.dram_tensor(
            name, shape, dt, kind="ExternalInput").ap()
        xT = dr("xT", [I, T_STEPS, BL], dt_x)      # host pre-transposed [i, t, b]
        w1t = dr("w1t", [I, 4 * H1], dt_w)         # Wih1.T, gate-reordered
        wh1t = dr("wh1t", [H1, 4 * H1], bf16)
        w2t = dr("w2t", [H2, 4 * H2], bf16)
        wh2t = dr("wh2t", [H2, 4 * H2], bf16)
        w3t = dr("w3t", [H2, 4 * H3], bf16)
        wh3ta = dr("wh3ta", [H3 + 1, 4 * H3], bf16)  # [Whh3.T ; b3]
        b1r = dr("b1r", [1, 4 * H1], dt_w)
        b2r = dr("b2r", [1, 4 * H2], dt_w)
        gball = dr("gball", [128, 10], f32)  # gamma cols 0:5, beta cols 5:10
        wlt = dr("wlt", [H3, 2], bf16)       # Wl.T
        blp = dr("blp", [2, 1], f32)         # bl as per-partition bias
        wd = dr("wd", [2, 1], bf16)          # Wl2[0]-Wl2[1] as column
        headc = dr("headc", [128, 1], f32)   # bl2[0]-bl2[1] replicated
        y = nc.dram_tensor("y", [BL, 2 * T_STEPS], f32,
                           kind="ExternalOutput").ap()

        with (
            tc.tile_pool(name="const", bufs=1) as const,
            tc.tile_pool(name="state", bufs=1) as state,
            tc.tile_pool(name="xin", bufs=3) as xin,
            tc.tile_pool(name="work", bufs=3) as work,
            tc.tile_pool(name="zt", bufs=3) as ztp,
            tc.tile_pool(name="g13p", bufs=1, space="PSUM") as g13p,
            tc.tile_pool(name="g3p", bufs=1, space="PSUM") as g3p,
            tc.tile_pool(name="g2p", bufs=1, space="PSUM") as g2p,
            tc.tile_pool(name="smp", bufs=1, space="PSUM") as smp,
            tc.tile_pool(name="tp1", bufs=1, space="PSUM") as tp1,
            tc.tile_pool(name="tp2", bufs=1, space="PSUM") as tp2,
        ):
            # ---------------- constants ----------------
            ident_b = const.tile([128, 128], bf16)
            make_identity(nc, ident_b)
            ones_row = const.tile([1, 128], dt_w)
            nc.vector.memset(ones_row.bitcast(f32), 1.0)
            ones128 = const.tile([128, 128], bf16)
            nc.vector.memset(ones128, 1.0)
            magic_t = const.tile([128, 5], u32)
            nc.vector.memset(magic_t, 0x5F3759DF)

            def load(name, shape, dt, src):
                t = const.tile(shape, dt, tag=name)
                nc.sync.dma_start(t[:], src)
                return t

            w1t_s = load("w1t", [128, 4 * H1], dt_w, w1t[:])
            wh1t_s = load("wh1t", [128, 2, 4 * H1], bf16,
                          wh1t.rearrange("(k p) n -> p k n", p=128))
            w2t_s = load("w2t", [128, 2, 4 * H2], bf16,
                         w2t.rearrange("(k p) n -> p k n", p=128))
            wh2t_s = load("wh2t", [128, 2, 4 * H2], bf16,
                          wh2t.rearrange("(k p) n -> p k n", p=128))
            w3t_s = load("w3t", [128, 2, 4 * H3], bf16,
                         w3t.rearrange("(k p) n -> p k n", p=128))
            wh3ta_s = load("wh3ta", [H3 + 1, 4 * H3], bf16, wh3ta[:])
            b1r_s = load("b1r", [1, 4 * H1], dt_w, b1r[:])
            b2r_s = load("b2r", [1, 4 * H2], dt_w, b2r[:])
            gball_s = load("gball", [128, 10], f32, gball[:])
            wlt_s = load("wlt", [H3, 2], bf16, wlt[:])
            blp_s = load("blp", [2, 1], f32, blp[:])
            wd_s = load("wd", [2, 1], bf16, wd[:])
            headc_s = load("headc", [128, 1], f32, headc[:])

            # ---------------- persistent state ----------------
            # c1 and c3 share one tile so one tanh covers both.
            c13 = state.tile([128, H1 + H3], bf16)
            c2 = state.tile([128, H2], bf16)
            h1T = state.tile([128, 2, 128], bf16)   # feat-part, batch-free
            h2T = state.tile([128, 2, 128], bf16)
            h3Ta = state.tile([H3 + 1, 128], f32)   # last row = ones (bias)
            dstrip = state.tile([128, T_STEPS], f32)  # head logit diffs
            out_sb = state.tile([128, 2, T_STEPS], f32)
            nc.vector.memset(c13, 0.0)
            nc.vector.memset(c2, 0.0)
            for tens in (h1T, h2T):
                nc.vector.memset(tens, 0.0)
            nc.vector.memset(h3Ta[0:H3, :], 0.0)
            nc.vector.memset(h3Ta[H3:H3 + 1, :], 1.0)

            XCH = 8  # x chunk length (steps per DMA)

            R = run if run is not None else T_STEPS
            NS = R + 7

            # per-superstep (mean, var) tiles: chunk c in 0-4 =
            # L1j0, L1j1, L2j0, L2j1, L3; [:, c, 0] = mean, [:, c, 1] = var
            mvq = {}

            def get_mv(i):
                if i not in mvq:
                    mvq[i] = work.tile([128, 5, 2], f32, tag="mv", bufs=4,
                                       name="mvt")
                return mvq[i]

            h3q = {}
            h3init = work.tile([H3 + 1, 128], bf16, tag="h3a", bufs=4,
                               name="h3init")
            nc.vector.memset(h3init[0:H3, :], 0.0)
            nc.vector.memset(h3init[H3:H3 + 1, :], 1.0)
            h3q[3] = h3init
            z2q = {}
            z1T_prev = z3T_prev = None
            sig2_prev = sig_3_prev = s_prev = tt_prev = None

            for s in range(NS):
                # L3 runs at t=s-4 (consumes z2T from superstep s-2) so the
                # BN-coefficient chain never sits on the L1<->L3 merged-ACT
                # critical path. Head runs at t=s-7.
                g13f = g3f = None
                if s < R:
                    g13f = g13p.tile([128, 1024], f32, tag="g13")
                if 4 <= s <= R + 3:
                    g3f = g3p.tile([128, 128], f32, tag="g3")

                do_l1 = s < R
                do_l3 = 4 <= s <= R + 3
                do_l2 = 1 <= s <= R          # L2 gates+cell for t=s-1
                do_l2tail = 2 <= s <= R + 1  # tanh/h/transpose/stats of prev
                do_z2 = 3 <= s <= R + 2      # z2 = chain(s-1) . h2T
                do_l3tail = 5 <= s <= R + 4  # L3 tanh/h/transpose/stats
                do_head = 7 <= s <= R + 6

                if do_l1:
                    ti = s % XCH
                    if ti == 0:
                        xT_sb = xin.tile([128, XCH, 128], dt_x, tag="x")
                        nc.sync.dma_start(xT_sb, xT[:, s:s + XCH, :])
                    # three output column chunks: [0:512) if, [512:768) o,
                    # [1024:1280) g  (PSUM-bank-contained)
                    chunks = [(L1_IF, 0, 512), (L1_O, 512, 256),
                              (L1_G, 768, 256)]
                    for dst, wsrc, width in chunks:
                        oc = g13f[:, dst:dst + width]
                        wc = slice(wsrc, wsrc + width)
                        nc.tensor.matmul(oc, ones_row, b1r_s[:, wc],
                                         start=True, stop=False)
                        nc.tensor.matmul(oc, xT_sb[:, ti, :], w1t_s[:, wc],
                                         start=False, stop=False)
                        for k in range(2):
                            nc.tensor.matmul(oc, h1T[:, k, :],
                                             wh1t_s[:, k, wc],
                                             start=False, stop=(k == 1))

                # ---------- L2 matmuls @ t=s-1 ----------
                # h-part first (deps ready at superstep start), z-part after
                # (z1T lands at the end of superstep s-1)
                if do_l2:
                    z1T = z1T_prev
                    g2 = g2p.tile([128, 4 * H2], f32, tag="g2")
                    for nj in range(2):
                        nn_ = slice(512 * nj, 512 * (nj + 1))
                        nc.tensor.matmul(g2[:, nn_], ones_row, b2r_s[:, nn_],
                                         start=True, stop=False,
                                         skip_group_check=True)
                        for k in range(2):
                            nc.tensor.matmul(g2[:, nn_], h2T[:, k, :],
                                             wh2t_s[:, k, nn_],
                                             start=False, stop=False,
                                             skip_group_check=True)
                    for nj in range(2):
                        nn_ = slice(512 * nj, 512 * (nj + 1))
                        for k in range(2):
                            nc.tensor.matmul(g2[:, nn_], z1T[:, k, :],
                                             w2t_s[:, k, nn_],
                                             start=False, stop=(k == 1),
                                             skip_group_check=True)

                # head first-layer matmul (dep z3T from superstep s-1)
                if do_head:
                    o1t = smp.tile([2, 128], f32, tag="sm")
                    nc.tensor.matmul(o1t, wlt_s, z3T_prev, start=True,
                                     stop=True)

                # ---------- L1 activations (L1-only tiles: the chain never
                # waits on L3) ----------
                sig_if = sig_o = tg1 = None
                if do_l1:
                    sig_if = work.tile([128, 512], bf16, tag="sgIF")
                    nc.scalar.activation(sig_if, g13f[:, 0:512], AF.Sigmoid)
                    tg1 = work.tile([128, 256], bf16, tag="tgA")
                    nc.scalar.activation(tg1, g13f[:, L1_G:L1_G + 256],
                                         AF.Tanh)
                    sig_o = work.tile([128, 256], bf16, tag="sgO")
                    nc.scalar.activation(sig_o, g13f[:, 512:768], AF.Sigmoid)

                # ---------- L1 cell math (DVE) ----------
                h1 = None
                if do_l1:
                    cn = work.tile([128, H1], bf16, tag="cnA")
                    tm = work.tile([128, H1], bf16, tag="tmA")
                    nc.vector.tensor_mul(cn, sig_if[:, 256:512], c13[:, 0:H1])
                    nc.vector.tensor_mul(tm, sig_if[:, 0:256], tg1)
                    nc.vector.tensor_add(c13[:, 0:H1], cn, tm)

                # L1 tanh(c1) + h1 first: tc1 must not queue behind tc2/tc3;
                # transpose+copy immediately (h1T closes the L1 recurrence)
                if do_l1:
                    tc1 = work.tile([128, H1], bf16, tag="tcA")
                    nc.scalar.activation(tc1, c13[:, 0:H1], AF.Tanh)
                    h1 = work.tile([128, H1], bf16, tag="hA")
                    nc.vector.tensor_mul(h1, sig_o, tc1)
                    h1T_ps = tp1.tile([128, 2, 128], bf16, tag="tp1")
                    for j in range(2):
                        nc.tensor.transpose(h1T_ps[:, j, :],
                                            h1[:, j * 128:(j + 1) * 128],
                                            ident_b)
                    nc.vector.tensor_copy(h1T, h1T_ps)

                # ---------- L2 tail of prev superstep: tanh/h/transpose ----
                if do_l2tail:
                    tc2 = work.tile([128, H2], bf16, tag="tcB")
                    nc.scalar.activation(tc2, c2, AF.Tanh)
                    h2 = work.tile([128, H2], bf16, tag="hB")
                    nc.vector.tensor_mul(h2, sig2_prev[:, 512:768], tc2)
                    h2T_ps = tp2.tile([128, 2, 128], bf16, tag="tp2")
                    for j in range(2):
                        nc.tensor.transpose(h2T_ps[:, j, :],
                                            h2[:, j * 128:(j + 1) * 128],
                                            ident_b)
                    nc.vector.tensor_copy(h2T, h2T_ps)

                # ---------- L3 tail of prev superstep ----------
                if do_l3tail:
                    tc3 = work.tile([128, H3], bf16, tag="tcC")
                    nc.scalar.activation(tc3, c13[:, H1:H1 + H3], AF.Tanh)
                    h3 = work.tile([128, H3], bf16, tag="hC")
                    nc.gpsimd.tensor_mul(h3, sig_3_prev[:, 64:96], tc3)
                    h3T_ps = smp.tile([H3, 128], bf16, tag="sm")
                    nc.tensor.transpose(h3T_ps, h3, ident_b)
                    h3aug = work.tile([H3 + 1, 128], bf16, tag="h3a", bufs=4)
                    nc.vector.tensor_copy(h3aug[0:H3, :], h3T_ps)
                    nc.vector.memset(h3aug[H3:H3 + 1, :], 1.0)
                    h3q[s - 1] = h3aug

                # ---------- L1 stats ----------
                if do_l1:
                    mv = get_mv(s)
                    st1 = work.tile([128, 2, 6], f32, tag="st1")
                    for j in range(2):
                        nc.vector.bn_stats(st1[:, j, :], h1T[:, j, :])
                        nc.vector.bn_aggr(mv[:, j, :], st1[:, j, :])

                # ---------- L2 tail stats (feed chain(s)) ----------
                if do_l2tail:
                    mv = get_mv(s)
                    st2 = work.tile([128, 2, 6], f32, tag="st2")
                    for j in range(2):
                        nc.vector.bn_stats(st2[:, j, :], h2T[:, j, :])
                        nc.vector.bn_aggr(mv[:, 2 + j, :], st2[:, j, :])

                # ---------- head relu + diff col (t=s-7) ----------
                if do_head:
                    t_out = s - 7
                    relu1 = work.tile([2, 128], bf16, tag="rl")
                    nc.vector.tensor_scalar(
                        out=relu1, in0=o1t, scalar1=blp_s, scalar2=0.0,
                        op0=OP.add, op1=OP.max)
                    dcol = smp.tile([128, 1], f32, tag="sm")
                    nc.tensor.matmul(dcol, relu1, wd_s, start=True, stop=True)
                    nc.vector.tensor_copy(dstrip[:, t_out:t_out + 1], dcol)

                # ---------- L3 tail stats (feed chain(s+1)) ----------
                if do_l3tail:
                    mv3 = get_mv(s + 1)
                    st3 = work.tile([H3, 6], f32, tag="st3")
                    nc.vector.bn_stats(st3, h3q[s - 1][0:H3, :])
                    nc.vector.bn_aggr(mv3[0:H3, 4, :], st3)

                # ---------- L3 gates + cell @ t=s-4 ----------
                if do_l3:
                    z2T_in = z2q[s - 2]
                    # L3 gate cols in gL3: ifo [0:96), g [96:128)
                    for dst, wsrc, width in [(L3_IFO, 0, 96), (L3_G, 96, 32)]:
                        oc = g3f[:, dst:dst + width]
                        wc = slice(wsrc, wsrc + width)
                        nc.tensor.matmul(oc, z2T_in[:, 0, :], w3t_s[:, 0, wc],
                                         start=True, stop=False,
                                         skip_group_check=True)
                        nc.tensor.matmul(oc, z2T_in[:, 1, :], w3t_s[:, 1, wc],
                                         start=False, stop=False,
                                         skip_group_check=True)
                        nc.tensor.matmul(oc, h3q[s - 1], wh3ta_s[:, wc],
                                         start=False, stop=True,
                                         skip_group_check=True)
                    sig_3 = work.tile([128, 96], bf16, tag="sg3")
                    nc.scalar.activation(sig_3, g3f[:, L3_IFO:L3_IFO + 96],
                                         AF.Sigmoid)
                    tg3 = work.tile([128, H3], bf16, tag="tgC")
                    nc.scalar.activation(tg3, g3f[:, L3_G:L3_G + H3],
                                         AF.Tanh)
                    cn3 = work.tile([128, H3], bf16, tag="cnC")
                    tm3 = work.tile([128, H3], bf16, tag="tmC")
                    nc.gpsimd.tensor_mul(cn3, sig_3[:, 32:64],
                                         c13[:, H1:H1 + H3])
                    nc.gpsimd.tensor_mul(tm3, sig_3[:, 0:32], tg3)
                    nc.gpsimd.tensor_add(c13[:, H1:H1 + H3], cn3, tm3)
                    sig_3_prev = sig_3

                if s <= R + 5:
                    # ---- BN coefficients from bn_aggr's (mean, var), all on
                    # DVE ([128,5] ops are ~65ns there and z-applies are DVE
                    # anyway): fast-inverse-sqrt with ONE Newton iteration,
                    # gamma/beta fold.  10 serial ops.
                    mv = get_mv(s)
                    t2 = work.tile([128, 5], f32, tag="t2")
                    u2 = work.tile([128, 5], f32, tag="u2")
                    y1 = work.tile([128, 5], f32, tag="y1")
                    ve = work.tile([128, 5], f32, tag="ve")
                    s_ = work.tile([128, 5], f32, tag="s_")
                    tt = work.tile([128, 5], f32, tag="tt")
                    nc.vector.tensor_scalar(
                        out=ve, in0=mv[:, :, 1], scalar1=EPS, scalar2=None,
                        op0=OP.add)
                    nc.vector.tensor_scalar(
                        out=t2.bitcast(u32), in0=ve.bitcast(u32),
                        scalar1=1, scalar2=None, op0=OP.arith_shift_right)
                    nc.vector.tensor_sub(y1.bitcast(u32), magic_t,
                                         t2.bitcast(u32))
                    nc.vector.tensor_mul(u2, y1, y1)
                    nc.vector.tensor_mul(t2, ve, u2)
                    nc.vector.tensor_scalar(out=u2, in0=t2,
                                            scalar1=-0.5, scalar2=1.5,
                                            op0=OP.mult, op1=OP.add)
                    nc.vector.tensor_mul(y1, y1, u2)
                    nc.vector.tensor_mul(s_, y1, gball_s[:, 0:5])
                    nc.vector.tensor_mul(u2, mv[:, :, 0], s_)
                    nc.vector.tensor_sub(tt, gball_s[:, 5:10], u2)

                # ---------- L2 sigmoid/tanh + cell update ----------
                if do_l2:
                    sig2 = work.tile([128, 3 * H2], bf16, tag="sigB")
                    nc.scalar.activation(sig2, g2[:, 0:768], AF.Sigmoid)
                    tg2 = work.tile([128, H2], bf16, tag="tgB")
                    nc.scalar.activation(tg2, g2[:, 768:1024], AF.Tanh)
                    cn2 = work.tile([128, H2], bf16, tag="cnB")
                    tm2 = work.tile([128, H2], bf16, tag="tmB")
                    nc.vector.tensor_mul(cn2, sig2[:, 256:512], c2)
                    nc.vector.tensor_mul(tm2, sig2[:, 0:256], tg2)
                    nc.vector.tensor_add(c2, cn2, tm2)
                    sig2_prev = sig2

                # ---- BN applies on the h-states saved last superstep ----
                if s < R:
                    z1T = ztp.tile([128, 2, 128], bf16, tag="z1")
                    for j in range(2):
                        nc.vector.tensor_scalar(
                            out=z1T[:, j, :], in0=h1T[:, j, :],
                            scalar1=s_[:, j:j + 1], scalar2=tt[:, j:j + 1],
                            op0=OP.mult, op1=OP.add)
                    z1T_prev = z1T
                if do_z2:
                    # chain(s-1) applied to h2T written by this superstep's
                    # tail == baseline's z2T(s-1)
                    z2T = ztp.tile([128, 2, 128], bf16, tag="z2")
                    for j in range(2):
                        nc.gpsimd.tensor_scalar(
                            out=z2T[:, j, :], in0=h2T[:, j, :],
                            scalar1=s_prev[:, 2 + j:3 + j],
                            scalar2=tt_prev[:, 2 + j:3 + j],
                            op0=OP.mult, op1=OP.add)
                    z2q[s - 1] = z2T
                if 6 <= s <= R + 5:
                    z3T = ztp.tile([H3, 128], bf16, tag="z3")
                    nc.gpsimd.tensor_scalar(
                        out=z3T, in0=h3q[s - 2][0:H3, :],
                        scalar1=s_[0:H3, 4:5], scalar2=tt[0:H3, 4:5],
                        op0=OP.mult, op1=OP.add)
                    z3T_prev = z3T
                if s <= R + 5:
                    s_prev, tt_prev = s_, tt
                for k in [k for k in h3q if k <= s - 2]:
                    del h3q[k]
                for k in [k for k in z2q if k <= s - 2]:
                    del z2q[k]

            # ---------- amortized head sigmoid over the whole strip ----------
            # out_sb[:, 0, :] = sigmoid(d + c), out_sb[:, 1, :] = 1 - that;
            # the DMA interleaves them into y's (t, class) column order.
            nc.scalar.activation(out_sb[:, 0, :], dstrip, AF.Sigmoid,
                                 bias=headc_s, scale=1.0)
            nc.vector.tensor_scalar(
                out=out_sb[:, 1, :], in0=out_sb[:, 0, :],
                scalar1=-1.0, scalar2=1.0, op0=OP.mult, op1=OP.add)

            y_tc = y.rearrange("b (t two) -> b t two", two=2)
            nc.sync.dma_start(y_tc[:, :, 0], out_sb[:, 0, :])
            nc.sync.dma_start(y_tc[:, :, 1], out_sb[:, 1, :])

    nc.compile()
    return nc


def _prep_host(inputs, np_w, np_x):
    gp1 = _gate_perm(H1)
    gp2 = _gate_perm(H2)
    gp3 = _gate_perm(H3)
    f = lambda a: np.ascontiguousarray(a, dtype=np.float32)

    import ml_dtypes
    bf = ml_dtypes.bfloat16
    w1t = f(inputs["Wih1"][gp1].T).astype(np_w)
    wh1t = f(inputs["Whh1"][gp1].T).astype(bf)
    w2t = f(inputs["Wih2"][gp2].T).astype(bf)
    wh2t = f(inputs["Whh2"][gp2].T).astype(bf)
    w3t = f(inputs["Wih3"][gp3].T).astype(bf)
    wh3t = f(inputs["Whh3"][gp3].T).astype(bf)
    b1 = f(inputs["bih1"] + inputs["bhh1"])[gp1][None, :]
    b2 = f(inputs["bih2"] + inputs["bhh2"])[gp2][None, :]
    b3 = f(inputs["bih3"] + inputs["bhh3"])[gp3][None, :]
    wh3ta = np.concatenate([wh3t, b3.astype(bf)], axis=0)

    def cols128(v):  # [256] -> [128, 2]
        return np.ascontiguousarray(v.reshape(2, 128).T, dtype=np.float32)

    gball = np.zeros((128, 10), np.float32)
    gball[:, 0:2] = cols128(f(inputs["g1"]))
    gball[:, 2:4] = cols128(f(inputs["g2"]))
    gball[0:H3, 4] = f(inputs["g3"])
    gball[:, 5:7] = cols128(f(inputs["b1"]))
    gball[:, 7:9] = cols128(f(inputs["b2"]))
    gball[0:H3, 9] = f(inputs["b3"])

    wlt = f(inputs["Wl"].T).astype(bf)
    blp = f(inputs["bl"])[:, None]
    wd = f(inputs["Wl2"][0] - inputs["Wl2"][1])[:, None].astype(bf)
    dc = float(inputs["bl2"][0] - inputs["bl2"][1])
    headc = np.full((128, 1), dc, np.float32)

    shared = dict(w1t=w1t, wh1t=wh1t, w2t=w2t, wh2t=wh2t, w3t=w3t,
                  wh3ta=wh3ta, b1r=b1, b2r=b2, gball=gball,
                  wlt=wlt, blp=blp, wd=wd, headc=headc)

    x = np.asarray(inputs["x"], dtype=np.float32)
    in_maps = []
    for c in range(NCORES):
        xc = x[c * BL:(c + 1) * BL]
        xTc = np.ascontiguousarray(
            xc[:, :T_STEPS, :].transpose(2, 1, 0)).astype(np_x)
        m = dict(shared)
        m["xT"] = xTc
        in_maps.append(m)
    return in_maps


def kernel(**inputs):
    import concourse.mybir as mybir
    from concourse import bass_utils

    dt_w = mybir.dt.float32r
    dt_x = mybir.dt.float32r
    np_w = np.float32
    np_x = np.float32

    key = ("v4", str(dt_w), str(dt_x), T_STEPS, T_RUN)
    if key not in _CACHE:
        _CACHE[key] = _build(dt_w, dt_x, run=T_RUN)
    nc = _CACHE[key]

    in_maps = _prep_host(inputs, np_w, np_x)
    res = bass_utils.run_bass_kernel_spmd(nc, in_maps,
                                          core_ids=list(range(NCORES)))
    out = np.empty((B, T_STEPS, 2), np.float32)
    for c in range(NCORES):
        out[c * BL:(c + 1) * BL] = res.results[c]["y"].reshape(BL, T_STEPS, 2)
    return out



# revision 54
# speedup vs baseline: 1.0248x; 1.0248x over previous
"""Trainium2 Bass kernel for nn_CryptoNet: 3-layer LSTM + per-step BatchNorm + 2-layer head.

Strategy: 8-way data parallel over batch (128 samples/core), zero cross-core
communication (BN uses per-shard batch stats; measured rel err vs full-batch
stats ~1.8e-3).

v5 changes over v4 (TimelineSim 1.836ms -> 1.558ms; rel err unchanged at
6.03e-3).  The superstep was latency-bound (all engines <60% busy), so the
work targets the L1 recurrence cycle and the in-order engine-queue coupling:
  - BN stats via per-chunk bn_stats/bn_aggr on the SBUF h.T copies (replaces
    8 scalar_tensor_tensor passes); the rsqrt Newton chain moved entirely to
    DVE where [128,5] ops cost ~65ns and feed the DVE z-applies directly.
  - L2 pipeline split across supersteps: gates+sigmoid+cell update at s, the
    tanh/h/transpose/copy/stats tail at the FRONT of s+1, so the next
    superstep's L1 sigmoid never head-blocks behind L2's late activations.
    L3 restructured the same way (tail one superstep later).
  - Separate PSUM tiles for L1 vs L3 gates (and separate ACT ops/tiles per
    layer): next-superstep L1 matmuls no longer wait for L3's late PSUM
    readers; tanh(c1) no longer waits on Pool's c3 cell math.
  - L1 chain ACT split: sigmoid(i,f) first (unblocks cell math), then
    sigmoid(o) / tanh(g) while DVE runs.
  - head: first-layer matmul issued at superstep top (z3T ready), relu on
    DVE, z2/z3 BN-applies on Pool (Pool cannot touch PSUM; all its operands
    are SBUF).
  - per-step sigmoid/1-p of the head amortized into one strip op at the end
    (unchanged from v4); cell state c in bf16 (unchanged from v4).
"""

import sys
import numpy as np

for p in ("/opt/trn_rl_repo", "/opt/trn_rl_repo/concourse"):
    if p not in sys.path:
        sys.path.insert(0, p)

B, T, I = 1024, 256, 128
T_STEPS = T  # override for small-scale testing
T_RUN = None  # loop steps; defaults to T_STEPS
H1, H2, H3 = 256, 256, 32
NCORES = 8
BL = B // NCORES  # local batch per core = 128
EPS = 1e-5

_CACHE = {}


def _gate_perm(H):
    # torch gate order (i, f, g, o) -> (i, f, o, g)
    idx = np.arange(4 * H)
    i, f, g, o = np.split(idx, 4)
    return np.concatenate([i, f, o, g])


def _build(dt_w, dt_x, run=None):
    import concourse.bass as bass
    import concourse.tile as tile
    import concourse.mybir as mybir
    from concourse import bacc
    from concourse.masks import make_identity

    f32 = mybir.dt.float32
    u32 = mybir.dt.uint32
    f32r = mybir.dt.float32r
    AF = mybir.ActivationFunctionType
    OP = mybir.AluOpType
    bf16 = mybir.dt.bfloat16

    nc = bacc.Bacc("TRN2", target_bir_lowering=False, debug=False,
                   num_devices=NCORES)

    # gL1 PSUM layout (f32 cols, 2 banks): if [0:512), o [512:768),
    # g [768:1024).  L3 gates live in their own 1-bank tile: ifo [0:96),
    # g [96:128) -- separate tiles so next-superstep L1 matmuls never wait
    # on L3's late ACT readers.
    L1_IF = 0        # 512 wide
    L1_O = 512       # 256 wide
    L1_G = 768       # 256 wide
    L3_IFO = 0       # 96 wide (in gL3)
    L3_G = 96        # 32 wide (in gL3)

    with tile.TileContext(nc) as tc:
        dr = lambda name, shape, dt: nc.dram_tensor(
            name, shape, dt, kind="ExternalInput").ap()
        xT = dr("xT", [I, T_STEPS, BL], dt_x)      # host pre-transposed [i, t, b]
        w1t = dr("w1t", [I, 4 * H1], dt_w)         # Wih1.T, gate-reordered
        wh1t = dr("wh1t", [H1, 4 * H1], bf16)
        w2t = dr("w2t", [H2, 4 * H2], bf16)
        wh2t = dr("wh2t", [H2, 4 * H2], bf16)
        w3t = dr("w3t", [H2, 4 * H3], bf16)
        wh3ta = dr("wh3ta", [H3 + 1, 4 * H3], bf16)  # [Whh3.T ; b3]
        b1r = dr("b1r", [1, 4 * H1], dt_w)
        b2r = dr("b2r", [1, 4 * H2], dt_w)
        gball = dr("gball", [128, 10], f32)  # gamma cols 0:5, beta cols 5:10
        wlt = dr("wlt", [H3, 2], bf16)       # Wl.T
        blp = dr("blp", [2, 1], f32)         # bl as per-partition bias
        wd = dr("wd", [2, 1], bf16)          # Wl2[0]-Wl2[1] as column
        headc = dr("headc", [128, 1], f32)   # bl2[0]-bl2[1] replicated
        y = nc.dram_tensor("y", [BL, 2 * T_STEPS], f32,
                           kind="ExternalOutput").ap()

        with (
            tc.tile_pool(name="const", bufs=1) as const,
            tc.tile_pool(name="state", bufs=1) as state,
            tc.tile_pool(name="xin", bufs=3) as xin,
            tc.tile_pool(name="work", bufs=3) as work,
            tc.tile_pool(name="zt", bufs=3) as ztp,
            tc.tile_pool(name="g13p", bufs=1, space="PSUM") as g13p,
            tc.tile_pool(name="g3p", bufs=1, space="PSUM") as g3p,
            tc.tile_pool(name="g2p", bufs=1, space="PSUM") as g2p,
            tc.tile_pool(name="smp", bufs=1, space="PSUM") as smp,
            tc.tile_pool(name="tp1", bufs=1, space="PSUM") as tp1,
            tc.tile_pool(name="tp2", bufs=1, space="PSUM") as tp2,
        ):
            # ---------------- constants ----------------
            ident_b = const.tile([128, 128], bf16)
            make_identity(nc, ident_b)
            ones_row = const.tile([1, 128], dt_w)
            nc.vector.memset(ones_row.bitcast(f32), 1.0)
            ones128 = const.tile([128, 128], bf16)
            nc.vector.memset(ones128, 1.0)
            magic_t = const.tile([128, 5], u32)
            nc.vector.memset(magic_t, 0x5F3759DF)

            def load(name, shape, dt, src):
                t = const.tile(shape, dt, tag=name)
                nc.sync.dma_start(t[:], src)
                return t

            w1t_s = load("w1t", [128, 4 * H1], dt_w, w1t[:])
            wh1t_s = load("wh1t", [128, 2, 4 * H1], bf16,
                          wh1t.rearrange("(k p) n -> p k n", p=128))
            w2t_s = load("w2t", [128, 2, 4 * H2], bf16,
                         w2t.rearrange("(k p) n -> p k n", p=128))
            wh2t_s = load("wh2t", [128, 2, 4 * H2], bf16,
                          wh2t.rearrange("(k p) n -> p k n", p=128))
            w3t_s = load("w3t", [128, 2, 4 * H3], bf16,
                         w3t.rearrange("(k p) n -> p k n", p=128))
            wh3ta_s = load("wh3ta", [H3 + 1, 4 * H3], bf16, wh3ta[:])
            b1r_s = load("b1r", [1, 4 * H1], dt_w, b1r[:])
            b2r_s = load("b2r", [1, 4 * H2], dt_w, b2r[:])
            gball_s = load("gball", [128, 10], f32, gball[:])
            wlt_s = load("wlt", [H3, 2], bf16, wlt[:])
            blp_s = load("blp", [2, 1], f32, blp[:])
            wd_s = load("wd", [2, 1], bf16, wd[:])
            headc_s = load("headc", [128, 1], f32, headc[:])

            # ---------------- persistent state ----------------
            # c1 and c3 share one tile so one tanh covers both.
            c13 = state.tile([128, H1 + H3], bf16)
            c2 = state.tile([128, H2], bf16)
            h1T = state.tile([128, 2, 128], bf16)   # feat-part, batch-free
            h2T = state.tile([128, 2, 128], bf16)
            h3Ta = state.tile([H3 + 1, 128], f32)   # last row = ones (bias)
            dstrip = state.tile([128, T_STEPS], f32)  # head logit diffs
            out_sb = state.tile([128, 2, T_STEPS], f32)
            nc.vector.memset(c13, 0.0)
            nc.vector.memset(c2, 0.0)
            for tens in (h1T, h2T):
                nc.vector.memset(tens, 0.0)
            nc.vector.memset(h3Ta[0:H3, :], 0.0)
            nc.vector.memset(h3Ta[H3:H3 + 1, :], 1.0)

            XCH = 8  # x chunk length (steps per DMA)

            R = run if run is not None else T_STEPS
            NS = R + 7

            # per-superstep (mean, var) tiles: chunk c in 0-4 =
            # L1j0, L1j1, L2j0, L2j1, L3; [:, c, 0] = mean, [:, c, 1] = var
            mvq = {}

            def get_mv(i):
                if i not in mvq:
                    mvq[i] = work.tile([128, 5, 2], f32, tag="mv", bufs=4,
                                       name="mvt")
                return mvq[i]

            h3q = {}
            h3init = work.tile([H3 + 1, 128], bf16, tag="h3a", bufs=4,
                               name="h3init")
            nc.vector.memset(h3init[0:H3, :], 0.0)
            nc.vector.memset(h3init[H3:H3 + 1, :], 1.0)
            h3q[3] = h3init
            z2q = {}
            z1T_prev = z3T_prev = None
            sig2_prev = sig_3_prev = s_prev = tt_prev = None

            for s in range(NS):
                # L3 runs at t=s-4 (consumes z2T from superstep s-2) so the
                # BN-coefficient chain never sits on the L1<->L3 merged-ACT
                # critical path. Head runs at t=s-7.
                g13f = g3f = None
                if s < R:
                    g13f = g13p.tile([128, 1024], f32, tag="g13")
                if 4 <= s <= R + 3:
                    g3f = g3p.tile([128, 128], f32, tag="g3")

                do_l1 = s < R
                do_l3 = 4 <= s <= R + 3
                do_l2 = 1 <= s <= R          # L2 gates+cell for t=s-1
                do_l2tail = 2 <= s <= R + 1  # tanh/h/transpose/stats of prev
                do_z2 = 3 <= s <= R + 2      # z2 = chain(s-1) . h2T
                do_l3tail = 5 <= s <= R + 4  # L3 tanh/h/transpose/stats
                do_head = 7 <= s <= R + 6

                if do_l1:
                    ti = s % XCH
                    if ti == 0:
                        xT_sb = xin.tile([128, XCH, 128], dt_x, tag="x")
                        nc.sync.dma_start(xT_sb, xT[:, s:s + XCH, :])
                    # three output column chunks: [0:512) if, [512:768) o,
                    # [1024:1280) g  (PSUM-bank-contained)
                    chunks = [(L1_IF, 0, 512), (L1_O, 512, 256),
                              (L1_G, 768, 256)]
                    for dst, wsrc, width in chunks:
                        oc = g13f[:, dst:dst + width]
                        wc = slice(wsrc, wsrc + width)
                        nc.tensor.matmul(oc, ones_row, b1r_s[:, wc],
                                         start=True, stop=False)
                        nc.tensor.matmul(oc, xT_sb[:, ti, :], w1t_s[:, wc],
                                         start=False, stop=False)
                        for k in range(2):
                            nc.tensor.matmul(oc, h1T[:, k, :],
                                             wh1t_s[:, k, wc],
                                             start=False, stop=(k == 1))

                # ---------- L2 matmuls @ t=s-1 ----------
                # h-part first (deps ready at superstep start), z-part after
                # (z1T lands at the end of superstep s-1)
                if do_l2:
                    z1T = z1T_prev
                    g2 = g2p.tile([128, 4 * H2], f32, tag="g2")
                    for nj in range(2):
                        nn_ = slice(512 * nj, 512 * (nj + 1))
                        nc.tensor.matmul(g2[:, nn_], ones_row, b2r_s[:, nn_],
                                         start=True, stop=False,
                                         skip_group_check=True)
                        for k in range(2):
                            nc.tensor.matmul(g2[:, nn_], h2T[:, k, :],
                                             wh2t_s[:, k, nn_],
                                             start=False, stop=False,
                                             skip_group_check=True)
                    for nj in range(2):
                        nn_ = slice(512 * nj, 512 * (nj + 1))
                        for k in range(2):
                            nc.tensor.matmul(g2[:, nn_], z1T[:, k, :],
                                             w2t_s[:, k, nn_],
                                             start=False, stop=(k == 1),
                                             skip_group_check=True)

                # head first-layer matmul (dep z3T from superstep s-1)
                if do_head:
                    o1t = smp.tile([2, 128], f32, tag="sm")
                    nc.tensor.matmul(o1t, wlt_s, z3T_prev, start=True,
                                     stop=True)

                # ---------- L1 activations (L1-only tiles: the chain never
                # waits on L3) ----------
                sig_if = sig_o = tg1 = None
                if do_l1:
                    sig_if = work.tile([128, 512], bf16, tag="sgIF")
                    nc.scalar.activation(sig_if, g13f[:, 0:512], AF.Sigmoid)
                    tg1 = work.tile([128, 256], bf16, tag="tgA")
                    nc.scalar.activation(tg1, g13f[:, L1_G:L1_G + 256],
                                         AF.Tanh)
                    sig_o = work.tile([128, 256], bf16, tag="sgO")
                    nc.scalar.activation(sig_o, g13f[:, 512:768], AF.Sigmoid)

                # ---------- L1 cell math (DVE) ----------
                h1 = None
                if do_l1:
                    cn = work.tile([128, H1], bf16, tag="cnA")
                    tm = work.tile([128, H1], bf16, tag="tmA")
                    nc.vector.tensor_mul(cn, sig_if[:, 256:512], c13[:, 0:H1])
                    nc.vector.tensor_mul(tm, sig_if[:, 0:256], tg1)
                    nc.vector.tensor_add(c13[:, 0:H1], cn, tm)

                # L1 tanh(c1) + h1 first: tc1 must not queue behind tc2/tc3;
                # transpose+copy immediately (h1T closes the L1 recurrence)
                if do_l1:
                    tc1 = work.tile([128, H1], bf16, tag="tcA")
                    nc.scalar.activation(tc1, c13[:, 0:H1], AF.Tanh)
                    h1 = work.tile([128, H1], bf16, tag="hA")
                    nc.vector.tensor_mul(h1, sig_o, tc1)
                    h1T_ps = tp1.tile([128, 2, 128], bf16, tag="tp1")
                    for j in range(2):
                        nc.tensor.transpose(h1T_ps[:, j, :],
                                            h1[:, j * 128:(j + 1) * 128],
                                            ident_b)
                    nc.vector.tensor_copy(h1T, h1T_ps)

                # ---------- L2 tail of prev superstep: tanh/h/transpose ----
                if do_l2tail:
                    tc2 = work.tile([128, H2], bf16, tag="tcB")
                    nc.scalar.activation(tc2, c2, AF.Tanh)
                    h2 = work.tile([128, H2], bf16, tag="hB")
                    nc.vector.tensor_mul(h2, sig2_prev[:, 512:768], tc2)
                    h2T_ps = tp2.tile([128, 2, 128], bf16, tag="tp2")
                    for j in range(2):
                        nc.tensor.transpose(h2T_ps[:, j, :],
                                            h2[:, j * 128:(j + 1) * 128],
                                            ident_b)
                    nc.vector.tensor_copy(h2T, h2T_ps)

                # ---------- L3 tail of prev superstep ----------
                if do_l3tail:
                    tc3 = work.tile([128, H3], bf16, tag="tcC")
                    nc.scalar.activation(tc3, c13[:, H1:H1 + H3], AF.Tanh)
                    h3 = work.tile([128, H3], bf16, tag="hC")
                    nc.gpsimd.tensor_mul(h3, sig_3_prev[:, 64:96], tc3)
                    h3T_ps = smp.tile([H3, 128], bf16, tag="sm")
                    nc.tensor.transpose(h3T_ps, h3, ident_b)
                    h3aug = work.tile([H3 + 1, 128], bf16, tag="h3a", bufs=4)
                    nc.scalar.copy(h3aug[0:H3, :], h3T_ps)
                    nc.gpsimd.memset(h3aug[H3:H3 + 1, :], 1.0)
                    h3q[s - 1] = h3aug

                # ---------- L1 stats ----------
                if do_l1:
                    mv = get_mv(s)
                    st1 = work.tile([128, 2, 6], f32, tag="st1")
                    for j in range(2):
                        nc.vector.bn_stats(st1[:, j, :], h1T[:, j, :])
                        nc.vector.bn_aggr(mv[:, j, :], st1[:, j, :])

                # ---------- L2 tail stats (feed chain(s)) ----------
                if do_l2tail:
                    mv = get_mv(s)
                    st2 = work.tile([128, 2, 6], f32, tag="st2")
                    for j in range(2):
                        nc.vector.bn_stats(st2[:, j, :], h2T[:, j, :])
                        nc.vector.bn_aggr(mv[:, 2 + j, :], st2[:, j, :])

                # ---------- head relu + diff col (t=s-7) ----------
                if do_head:
                    t_out = s - 7
                    relu1 = work.tile([2, 128], bf16, tag="rl")
                    nc.scalar.activation(relu1, o1t, AF.Relu, bias=blp_s)
                    dcol = smp.tile([128, 1], f32, tag="sm")
                    nc.tensor.matmul(dcol, relu1, wd_s, start=True, stop=True)
                    nc.vector.tensor_copy(dstrip[:, t_out:t_out + 1], dcol)

                # ---------- L3 tail stats (feed chain(s+1)) ----------
                if do_l3tail:
                    mv3 = get_mv(s + 1)
                    st3 = work.tile([H3, 6], f32, tag="st3")
                    nc.vector.bn_stats(st3, h3q[s - 1][0:H3, :])
                    nc.vector.bn_aggr(mv3[0:H3, 4, :], st3)

                # ---------- L3 gates + cell @ t=s-4 ----------
                if do_l3:
                    z2T_in = z2q[s - 2]
                    # L3 gate cols in gL3: ifo [0:96), g [96:128)
                    for dst, wsrc, width in [(L3_IFO, 0, 96), (L3_G, 96, 32)]:
                        oc = g3f[:, dst:dst + width]
                        wc = slice(wsrc, wsrc + width)
                        nc.tensor.matmul(oc, z2T_in[:, 0, :], w3t_s[:, 0, wc],
                                         start=True, stop=False,
                                         skip_group_check=True)
                        nc.tensor.matmul(oc, z2T_in[:, 1, :], w3t_s[:, 1, wc],
                                         start=False, stop=False,
                                         skip_group_check=True)
                        nc.tensor.matmul(oc, h3q[s - 1], wh3ta_s[:, wc],
                                         start=False, stop=True,
                                         skip_group_check=True)
                    sig_3 = work.tile([128, 96], bf16, tag="sg3")
                    nc.scalar.activation(sig_3, g3f[:, L3_IFO:L3_IFO + 96],
                                         AF.Sigmoid)
                    tg3 = work.tile([128, H3], bf16, tag="tgC")
                    nc.scalar.activation(tg3, g3f[:, L3_G:L3_G + H3],
                                         AF.Tanh)
                    cn3 = work.tile([128, H3], bf16, tag="cnC")
                    tm3 = work.tile([128, H3], bf16, tag="tmC")
                    nc.gpsimd.tensor_mul(cn3, sig_3[:, 32:64],
                                         c13[:, H1:H1 + H3])
                    nc.gpsimd.tensor_mul(tm3, sig_3[:, 0:32], tg3)
                    nc.gpsimd.tensor_add(c13[:, H1:H1 + H3], cn3, tm3)
                    sig_3_prev = sig_3

                if s <= R + 5:
                    # ---- BN coefficients from bn_aggr's (mean, var), all on
                    # DVE ([128,5] ops are ~65ns there and z-applies are DVE
                    # anyway): fast-inverse-sqrt with ONE Newton iteration,
                    # gamma/beta fold.  10 serial ops.
                    mv = get_mv(s)
                    t2 = work.tile([128, 5], f32, tag="t2")
                    u2 = work.tile([128, 5], f32, tag="u2")
                    y1 = work.tile([128, 5], f32, tag="y1")
                    ve = work.tile([128, 5], f32, tag="ve")
                    s_ = work.tile([128, 5], f32, tag="s_")
                    tt = work.tile([128, 5], f32, tag="tt")
                    nc.vector.tensor_scalar(
                        out=ve, in0=mv[:, :, 1], scalar1=EPS, scalar2=None,
                        op0=OP.add)
                    nc.vector.tensor_scalar(
                        out=t2.bitcast(u32), in0=ve.bitcast(u32),
                        scalar1=1, scalar2=None, op0=OP.arith_shift_right)
                    nc.vector.tensor_sub(y1.bitcast(u32), magic_t,
                                         t2.bitcast(u32))
                    nc.vector.tensor_mul(u2, y1, y1)
                    nc.vector.tensor_mul(t2, ve, u2)
                    nc.vector.tensor_scalar(out=u2, in0=t2,
                                            scalar1=-0.5, scalar2=1.5,
                                            op0=OP.mult, op1=OP.add)
                    nc.vector.tensor_mul(y1, y1, u2)
                    nc.vector.tensor_mul(s_, y1, gball_s[:, 0:5])
                    nc.vector.tensor_mul(u2, mv[:, :, 0], s_)
                    nc.vector.tensor_sub(tt, gball_s[:, 5:10], u2)

                # ---------- L2 sigmoid/tanh + cell update ----------
                if do_l2:
                    # split by PSUM bank: cell math waits only the i,f bank
                    sig2 = work.tile([128, 3 * H2], bf16, tag="sigB")
                    nc.scalar.activation(sig2[:, 0:512], g2[:, 0:512],
                                         AF.Sigmoid)
                    nc.scalar.activation(sig2[:, 512:768], g2[:, 512:768],
                                         AF.Sigmoid)
                    tg2 = work.tile([128, H2], bf16, tag="tgB")
                    nc.scalar.activation(tg2, g2[:, 768:1024], AF.Tanh)
                    cn2 = work.tile([128, H2], bf16, tag="cnB")
                    tm2 = work.tile([128, H2], bf16, tag="tmB")
                    nc.vector.tensor_mul(cn2, sig2[:, 256:512], c2)
                    nc.vector.tensor_mul(tm2, sig2[:, 0:256], tg2)
                    nc.vector.tensor_add(c2, cn2, tm2)
                    sig2_prev = sig2

                # ---- BN applies on the h-states saved last superstep ----
                if s < R:
                    z1T = ztp.tile([128, 2, 128], bf16, tag="z1")
                    for j in range(2):
                        nc.vector.tensor_scalar(
                            out=z1T[:, j, :], in0=h1T[:, j, :],
                            scalar1=s_[:, j:j + 1], scalar2=tt[:, j:j + 1],
                            op0=OP.mult, op1=OP.add)
                    z1T_prev = z1T
                if do_z2:
                    # chain(s-1) applied to h2T written by this superstep's
                    # tail == baseline's z2T(s-1)
                    z2T = ztp.tile([128, 2, 128], bf16, tag="z2")
                    for j in range(2):
                        nc.gpsimd.tensor_scalar(
                            out=z2T[:, j, :], in0=h2T[:, j, :],
                            scalar1=s_prev[:, 2 + j:3 + j],
                            scalar2=tt_prev[:, 2 + j:3 + j],
                            op0=OP.mult, op1=OP.add)
                    z2q[s - 1] = z2T
                if 6 <= s <= R + 5:
                    z3T = ztp.tile([H3, 128], bf16, tag="z3")
                    nc.gpsimd.tensor_scalar(
                        out=z3T, in0=h3q[s - 2][0:H3, :],
                        scalar1=s_[0:H3, 4:5], scalar2=tt[0:H3, 4:5],
                        op0=OP.mult, op1=OP.add)
                    z3T_prev = z3T
                if s <= R + 5:
                    s_prev, tt_prev = s_, tt
                for k in [k for k in h3q if k <= s - 2]:
                    del h3q[k]
                for k in [k for k in z2q if k <= s - 2]:
                    del z2q[k]

            # ---------- amortized head sigmoid over the whole strip ----------
            # out_sb[:, 0, :] = sigmoid(d + c), out_sb[:, 1, :] = 1 - that;
            # the DMA interleaves them into y's (t, class) column order.
            nc.scalar.activation(out_sb[:, 0, :], dstrip, AF.Sigmoid,
                                 bias=headc_s, scale=1.0)
            nc.vector.tensor_scalar(
                out=out_sb[:, 1, :], in0=out_sb[:, 0, :],
                scalar1=-1.0, scalar2=1.0, op0=OP.mult, op1=OP.add)

            y_tc = y.rearrange("b (t two) -> b t two", two=2)
            nc.sync.dma_start(y_tc[:, :, 0], out_sb[:, 0, :])
            nc.sync.dma_start(y_tc[:, :, 1], out_sb[:, 1, :])

    nc.compile()
    return nc


def _prep_host(inputs, np_w, np_x):
    gp1 = _gate_perm(H1)
    gp2 = _gate_perm(H2)
    gp3 = _gate_perm(H3)
    f = lambda a: np.ascontiguousarray(a, dtype=np.float32)

    import ml_dtypes
    bf = ml_dtypes.bfloat16
    w1t = f(inputs["Wih1"][gp1].T).astype(np_w)
    wh1t = f(inputs["Whh1"][gp1].T).astype(bf)
    w2t = f(inputs["Wih2"][gp2].T).astype(bf)
    wh2t = f(inputs["Whh2"][gp2].T).astype(bf)
    w3t = f(inputs["Wih3"][gp3].T).astype(bf)
    wh3t = f(inputs["Whh3"][gp3].T).astype(bf)
    b1 = f(inputs["bih1"] + inputs["bhh1"])[gp1][None, :]
    b2 = f(inputs["bih2"] + inputs["bhh2"])[gp2][None, :]
    b3 = f(inputs["bih3"] + inputs["bhh3"])[gp3][None, :]
    wh3ta = np.concatenate([wh3t, b3.astype(bf)], axis=0)

    def cols128(v):  # [256] -> [128, 2]
        return np.ascontiguousarray(v.reshape(2, 128).T, dtype=np.float32)

    gball = np.zeros((128, 10), np.float32)
    gball[:, 0:2] = cols128(f(inputs["g1"]))
    gball[:, 2:4] = cols128(f(inputs["g2"]))
    gball[0:H3, 4] = f(inputs["g3"])
    gball[:, 5:7] = cols128(f(inputs["b1"]))
    gball[:, 7:9] = cols128(f(inputs["b2"]))
    gball[0:H3, 9] = f(inputs["b3"])

    wlt = f(inputs["Wl"].T).astype(bf)
    blp = f(inputs["bl"])[:, None]
    wd = f(inputs["Wl2"][0] - inputs["Wl2"][1])[:, None].astype(bf)
    dc = float(inputs["bl2"][0] - inputs["bl2"][1])
    headc = np.full((128, 1), dc, np.float32)

    shared = dict(w1t=w1t, wh1t=wh1t, w2t=w2t, wh2t=wh2t, w3t=w3t,
                  wh3ta=wh3ta, b1r=b1, b2r=b2, gball=gball,
                  wlt=wlt, blp=blp, wd=wd, headc=headc)

    x = np.asarray(inputs["x"], dtype=np.float32)
    in_maps = []
    for c in range(NCORES):
        xc = x[c * BL:(c + 1) * BL]
        xTc = np.ascontiguousarray(
            xc[:, :T_STEPS, :].transpose(2, 1, 0)).astype(np_x)
        m = dict(shared)
        m["xT"] = xTc
        in_maps.append(m)
    return in_maps


def kernel(**inputs):
    import concourse.mybir as mybir
    from concourse import bass_utils

    dt_w = mybir.dt.float32r
    dt_x = mybir.dt.float32r
    np_w = np.float32
    np_x = np.float32

    key = ("v4", str(dt_w), str(dt_x), T_STEPS, T_RUN)
    if key not in _CACHE:
        _CACHE[key] = _build(dt_w, dt_x, run=T_RUN)
    nc = _CACHE[key]

    in_maps = _prep_host(inputs, np_w, np_x)
    res = bass_utils.run_bass_kernel_spmd(nc, in_maps,
                                          core_ids=list(range(NCORES)))
    out = np.empty((B, T_STEPS, 2), np.float32)
    for c in range(NCORES):
        out[c * BL:(c + 1) * BL] = res.results[c]["y"].reshape(BL, T_STEPS, 2)
    return out



# revision 60
# speedup vs baseline: 1.0285x; 1.0036x over previous
"""Trainium2 Bass kernel for nn_CryptoNet: 3-layer LSTM + per-step BatchNorm + 2-layer head.

Strategy: 8-way data parallel over batch (128 samples/core), zero cross-core
communication (BN uses per-shard batch stats; measured rel err vs full-batch
stats ~1.8e-3).

v5 changes over v4 (TimelineSim 1.836ms -> 1.521ms; rel err unchanged at
6.03e-3).  The superstep was latency-bound (all engines <60% busy), so the
work targets the L1 recurrence cycle and the in-order engine-queue coupling:
  - BN stats via per-chunk bn_stats/bn_aggr on the SBUF h.T copies (replaces
    8 scalar_tensor_tensor passes); the rsqrt Newton chain moved entirely to
    DVE where [128,5] ops cost ~65ns and feed the DVE z-applies directly.
  - L2 pipeline split across supersteps: gates+sigmoid+cell update at s, the
    tanh/h/transpose/copy/stats tail at the FRONT of s+1, so the next
    superstep's L1 sigmoid never head-blocks behind L2's late activations.
    L3 restructured the same way (tail one superstep later).
  - Separate PSUM tiles for L1 vs L3 gates (and separate ACT ops/tiles per
    layer): next-superstep L1 matmuls no longer wait for L3's late PSUM
    readers; tanh(c1) no longer waits on Pool's c3 cell math.
  - L1 chain ACT split: sigmoid(i,f) first (unblocks cell math), then
    sigmoid(o) / tanh(g) while DVE runs.
  - head: first-layer matmul issued at superstep top (z3T ready), relu on
    DVE, z2/z3 BN-applies on Pool (Pool cannot touch PSUM; all its operands
    are SBUF).
  - per-step sigmoid/1-p of the head amortized into one strip op at the end
    (unchanged from v4); cell state c in bf16 (unchanged from v4).
  - sig2 split by PSUM bank (cell math waits only the i,f bank); h3aug
    copy + head relu on ACT, h3aug ones-row memset on Pool (off the DVE
    conveyor, which is the serial resource ~2.9us/superstep).
"""

import sys
import numpy as np

for p in ("/opt/trn_rl_repo", "/opt/trn_rl_repo/concourse"):
    if p not in sys.path:
        sys.path.insert(0, p)

B, T, I = 1024, 256, 128
T_STEPS = T  # override for small-scale testing
T_RUN = None  # loop steps; defaults to T_STEPS
H1, H2, H3 = 256, 256, 32
NCORES = 8
BL = B // NCORES  # local batch per core = 128
EPS = 1e-5

_CACHE = {}


def _gate_perm(H):
    # torch gate order (i, f, g, o) -> (i, f, o, g)
    idx = np.arange(4 * H)
    i, f, g, o = np.split(idx, 4)
    return np.concatenate([i, f, o, g])


def _build(dt_w, dt_x, run=None):
    import concourse.bass as bass
    import concourse.tile as tile
    import concourse.mybir as mybir
    from concourse import bacc
    from concourse.masks import make_identity

    f32 = mybir.dt.float32
    u32 = mybir.dt.uint32
    f32r = mybir.dt.float32r
    AF = mybir.ActivationFunctionType
    OP = mybir.AluOpType
    bf16 = mybir.dt.bfloat16

    nc = bacc.Bacc("TRN2", target_bir_lowering=False, debug=False,
                   num_devices=NCORES)

    # gL1 PSUM layout (f32 cols, 2 banks): if [0:512), o [512:768),
    # g [768:1024).  L3 gates live in their own 1-bank tile: ifo [0:96),
    # g [96:128) -- separate tiles so next-superstep L1 matmuls never wait
    # on L3's late ACT readers.
    L1_IF = 0        # 512 wide
    L1_O = 512       # 256 wide
    L1_G = 768       # 256 wide
    L3_IFO = 0       # 96 wide (in gL3)
    L3_G = 96        # 32 wide (in gL3)

    with tile.TileContext(nc) as tc:
        dr = lambda name, shape, dt: nc.dram_tensor(
            name, shape, dt, kind="ExternalInput").ap()
        xT = dr("xT", [I, T_STEPS, BL], dt_x)      # host pre-transposed [i, t, b]
        w1t = dr("w1t", [I, 4 * H1], dt_w)         # Wih1.T, gate-reordered
        wh1t = dr("wh1t", [H1, 4 * H1], bf16)
        w2t = dr("w2t", [H2, 4 * H2], bf16)
        wh2t = dr("wh2t", [H2, 4 * H2], bf16)
        w3t = dr("w3t", [H2, 4 * H3], bf16)
        wh3ta = dr("wh3ta", [H3 + 1, 4 * H3], bf16)  # [Whh3.T ; b3]
        b1r = dr("b1r", [1, 4 * H1], dt_w)
        b2r = dr("b2r", [1, 4 * H2], dt_w)
        gball = dr("gball", [128, 10], f32)  # gamma cols 0:5, beta cols 5:10
        wlt = dr("wlt", [H3, 2], bf16)       # Wl.T
        blp = dr("blp", [2, 1], f32)         # bl as per-partition bias
        wd = dr("wd", [2, 1], bf16)          # Wl2[0]-Wl2[1] as column
        headc = dr("headc", [128, 1], f32)   # bl2[0]-bl2[1] replicated
        y = nc.dram_tensor("y", [BL, 2 * T_STEPS], f32,
                           kind="ExternalOutput").ap()

        with (
            tc.tile_pool(name="const", bufs=1) as const,
            tc.tile_pool(name="state", bufs=1) as state,
            tc.tile_pool(name="xin", bufs=3) as xin,
            tc.tile_pool(name="work", bufs=3) as work,
            tc.tile_pool(name="zt", bufs=3) as ztp,
            tc.tile_pool(name="g13p", bufs=1, space="PSUM") as g13p,
            tc.tile_pool(name="g3p", bufs=1, space="PSUM") as g3p,
            tc.tile_pool(name="g2p", bufs=1, space="PSUM") as g2p,
            tc.tile_pool(name="smp", bufs=1, space="PSUM") as smp,
            tc.tile_pool(name="tp1", bufs=1, space="PSUM") as tp1,
            tc.tile_pool(name="tp2", bufs=1, space="PSUM") as tp2,
        ):
            # ---------------- constants ----------------
            ident_b = const.tile([128, 128], bf16)
            make_identity(nc, ident_b)
            ones_row = const.tile([1, 128], dt_w)
            nc.vector.memset(ones_row.bitcast(f32), 1.0)
            ones128 = const.tile([128, 128], bf16)
            nc.vector.memset(ones128, 1.0)
            magic_t = const.tile([128, 5], u32)
            nc.vector.memset(magic_t, 0x5F3759DF)

            def load(name, shape, dt, src):
                t = const.tile(shape, dt, tag=name)
                nc.sync.dma_start(t[:], src)
                return t

            w1t_s = load("w1t", [128, 4 * H1], dt_w, w1t[:])
            wh1t_s = load("wh1t", [128, 2, 4 * H1], bf16,
                          wh1t.rearrange("(k p) n -> p k n", p=128))
            w2t_s = load("w2t", [128, 2, 4 * H2], bf16,
                         w2t.rearrange("(k p) n -> p k n", p=128))
            wh2t_s = load("wh2t", [128, 2, 4 * H2], bf16,
                          wh2t.rearrange("(k p) n -> p k n", p=128))
            w3t_s = load("w3t", [128, 2, 4 * H3], bf16,
                         w3t.rearrange("(k p) n -> p k n", p=128))
            wh3ta_s = load("wh3ta", [H3 + 1, 4 * H3], bf16, wh3ta[:])
            b1r_s = load("b1r", [1, 4 * H1], dt_w, b1r[:])
            b2r_s = load("b2r", [1, 4 * H2], dt_w, b2r[:])
            gball_s = load("gball", [128, 10], f32, gball[:])
            wlt_s = load("wlt", [H3, 2], bf16, wlt[:])
            blp_s = load("blp", [2, 1], f32, blp[:])
            wd_s = load("wd", [2, 1], bf16, wd[:])
            headc_s = load("headc", [128, 1], f32, headc[:])

            # ---------------- persistent state ----------------
            # c1 and c3 share one tile so one tanh covers both.
            c13 = state.tile([128, H1 + H3], bf16)
            c2 = state.tile([128, H2], bf16)
            h1T = state.tile([128, 2, 128], bf16)   # feat-part, batch-free
            h2T = state.tile([128, 2, 128], bf16)
            h3Ta = state.tile([H3 + 1, 128], f32)   # last row = ones (bias)
            dstrip = state.tile([128, T_STEPS], f32)  # head logit diffs
            out_sb = state.tile([128, 2, T_STEPS], f32)
            nc.vector.memset(c13, 0.0)
            nc.vector.memset(c2, 0.0)
            for tens in (h1T, h2T):
                nc.vector.memset(tens, 0.0)
            nc.vector.memset(h3Ta[0:H3, :], 0.0)
            nc.vector.memset(h3Ta[H3:H3 + 1, :], 1.0)

            XCH = 8  # x chunk length (steps per DMA)

            R = run if run is not None else T_STEPS
            NS = R + 7

            # per-superstep (mean, var) tiles: chunk c in 0-4 =
            # L1j0, L1j1, L2j0, L2j1, L3; [:, c, 0] = mean, [:, c, 1] = var
            mvq = {}

            def get_mv(i):
                if i not in mvq:
                    mvq[i] = work.tile([128, 5, 2], f32, tag="mv", bufs=4,
                                       name="mvt")
                return mvq[i]

            h3q = {}
            h3init = work.tile([H3 + 1, 128], bf16, tag="h3a", bufs=4,
                               name="h3init")
            nc.vector.memset(h3init[0:H3, :], 0.0)
            nc.vector.memset(h3init[H3:H3 + 1, :], 1.0)
            h3q[3] = h3init
            z2q = {}
            z1T_prev = z3T_prev = None
            sig2_prev = sig_3_prev = s_prev = tt_prev = None

            for s in range(NS):
                # L3 runs at t=s-4 (consumes z2T from superstep s-2) so the
                # BN-coefficient chain never sits on the L1<->L3 merged-ACT
                # critical path. Head runs at t=s-7.
                g13f = g3f = None
                if s < R:
                    g13f = g13p.tile([128, 1024], f32, tag="g13")
                if 4 <= s <= R + 3:
                    g3f = g3p.tile([128, 128], f32, tag="g3")

                do_l1 = s < R
                do_l3 = 4 <= s <= R + 3
                do_l2 = 1 <= s <= R          # L2 gates+cell for t=s-1
                do_l2tail = 2 <= s <= R + 1  # tanh/h/transpose/stats of prev
                do_z2 = 3 <= s <= R + 2      # z2 = chain(s-1) . h2T
                do_l3tail = 5 <= s <= R + 4  # L3 tanh/h/transpose/stats
                do_head = 7 <= s <= R + 6

                if do_l1:
                    ti = s % XCH
                    if ti == 0:
                        xT_sb = xin.tile([128, XCH, 128], dt_x, tag="x")
                        nc.sync.dma_start(xT_sb, xT[:, s:s + XCH, :])
                    # three output column chunks: [0:512) if, [512:768) o,
                    # [1024:1280) g  (PSUM-bank-contained)
                    chunks = [(L1_IF, 0, 512), (L1_O, 512, 256),
                              (L1_G, 768, 256)]
                    for dst, wsrc, width in chunks:
                        oc = g13f[:, dst:dst + width]
                        wc = slice(wsrc, wsrc + width)
                        nc.tensor.matmul(oc, ones_row, b1r_s[:, wc],
                                         start=True, stop=False)
                        nc.tensor.matmul(oc, xT_sb[:, ti, :], w1t_s[:, wc],
                                         start=False, stop=False)
                        for k in range(2):
                            nc.tensor.matmul(oc, h1T[:, k, :],
                                             wh1t_s[:, k, wc],
                                             start=False, stop=(k == 1))

                # ---------- L2 matmuls @ t=s-1 ----------
                # h-part first (deps ready at superstep start), z-part after
                # (z1T lands at the end of superstep s-1)
                if do_l2:
                    z1T = z1T_prev
                    g2 = g2p.tile([128, 4 * H2], f32, tag="g2")
                    for nj in range(2):
                        nn_ = slice(512 * nj, 512 * (nj + 1))
                        nc.tensor.matmul(g2[:, nn_], ones_row, b2r_s[:, nn_],
                                         start=True, stop=False,
                                         skip_group_check=True)
                        for k in range(2):
                            nc.tensor.matmul(g2[:, nn_], h2T[:, k, :],
                                             wh2t_s[:, k, nn_],
                                             start=False, stop=False,
                                             skip_group_check=True)
                    for nj in range(2):
                        nn_ = slice(512 * nj, 512 * (nj + 1))
                        for k in range(2):
                            nc.tensor.matmul(g2[:, nn_], z1T[:, k, :],
                                             w2t_s[:, k, nn_],
                                             start=False, stop=(k == 1),
                                             skip_group_check=True)

                # head first-layer matmul (dep z3T from superstep s-1)
                if do_head:
                    o1t = smp.tile([2, 128], f32, tag="sm")
                    nc.tensor.matmul(o1t, wlt_s, z3T_prev, start=True,
                                     stop=True)

                # ---------- L1 activations (L1-only tiles: the chain never
                # waits on L3) ----------
                sig_if = sig_o = tg1 = None
                if do_l1:
                    sig_if = work.tile([128, 512], bf16, tag="sgIF")
                    nc.scalar.activation(sig_if, g13f[:, 0:512], AF.Sigmoid)
                    tg1 = work.tile([128, 256], bf16, tag="tgA")
                    nc.scalar.activation(tg1, g13f[:, L1_G:L1_G + 256],
                                         AF.Tanh)
                    sig_o = work.tile([128, 256], bf16, tag="sgO")
                    nc.scalar.activation(sig_o, g13f[:, 512:768], AF.Sigmoid)

                # ---------- L1 cell math (DVE) ----------
                h1 = None
                if do_l1:
                    cn = work.tile([128, H1], bf16, tag="cnA")
                    tm = work.tile([128, H1], bf16, tag="tmA")
                    nc.vector.tensor_mul(cn, sig_if[:, 256:512], c13[:, 0:H1])
                    nc.vector.tensor_mul(tm, sig_if[:, 0:256], tg1)
                    nc.vector.tensor_add(c13[:, 0:H1], cn, tm)

                # ---------- L2 tail of prev superstep: tanh/h/transpose ----
                if do_l2tail:
                    tc2 = work.tile([128, H2], bf16, tag="tcB")
                    nc.scalar.activation(tc2, c2, AF.Tanh)
                    h2 = work.tile([128, H2], bf16, tag="hB")
                    nc.vector.tensor_mul(h2, sig2_prev[:, 512:768], tc2)
                    h2T_ps = tp2.tile([128, 2, 128], bf16, tag="tp2")
                    for j in range(2):
                        nc.tensor.transpose(h2T_ps[:, j, :],
                                            h2[:, j * 128:(j + 1) * 128],
                                            ident_b)
                    nc.vector.tensor_copy(h2T, h2T_ps)

                # L1 tanh(c1) + h1 (after tc2: tc2's dep is ready at superstep
                # start; tc1 is dep-bound anyway);
                # transpose+copy immediately (h1T closes the L1 recurrence)
                if do_l1:
                    tc1 = work.tile([128, H1], bf16, tag="tcA")
                    nc.scalar.activation(tc1, c13[:, 0:H1], AF.Tanh)
                    h1 = work.tile([128, H1], bf16, tag="hA")
                    nc.vector.tensor_mul(h1, sig_o, tc1)
                    h1T_ps = tp1.tile([128, 2, 128], bf16, tag="tp1")
                    for j in range(2):
                        nc.tensor.transpose(h1T_ps[:, j, :],
                                            h1[:, j * 128:(j + 1) * 128],
                                            ident_b)
                    nc.vector.tensor_copy(h1T, h1T_ps)

                # ---------- L3 tail of prev superstep ----------
                if do_l3tail:
                    tc3 = work.tile([128, H3], bf16, tag="tcC")
                    nc.scalar.activation(tc3, c13[:, H1:H1 + H3], AF.Tanh)
                    h3 = work.tile([128, H3], bf16, tag="hC")
                    nc.gpsimd.tensor_mul(h3, sig_3_prev[:, 64:96], tc3)
                    h3T_ps = smp.tile([H3, 128], bf16, tag="sm")
                    nc.tensor.transpose(h3T_ps, h3, ident_b)
                    h3aug = work.tile([H3 + 1, 128], bf16, tag="h3a", bufs=4)
                    nc.scalar.copy(h3aug[0:H3, :], h3T_ps)
                    nc.gpsimd.memset(h3aug[H3:H3 + 1, :], 1.0)
                    h3q[s - 1] = h3aug

                # ---------- L1 stats ----------
                if do_l1:
                    mv = get_mv(s)
                    st1 = work.tile([128, 2, 6], f32, tag="st1")
                    for j in range(2):
                        nc.vector.bn_stats(st1[:, j, :], h1T[:, j, :])
                        nc.vector.bn_aggr(mv[:, j, :], st1[:, j, :])

                # ---------- L2 tail stats (feed chain(s)) ----------
                if do_l2tail:
                    mv = get_mv(s)
                    st2 = work.tile([128, 2, 6], f32, tag="st2")
                    for j in range(2):
                        nc.vector.bn_stats(st2[:, j, :], h2T[:, j, :])
                        nc.vector.bn_aggr(mv[:, 2 + j, :], st2[:, j, :])

                # ---------- head relu + diff col (t=s-7) ----------
                if do_head:
                    t_out = s - 7
                    relu1 = work.tile([2, 128], bf16, tag="rl")
                    nc.scalar.activation(relu1, o1t, AF.Relu, bias=blp_s)
                    dcol = smp.tile([128, 1], f32, tag="sm")
                    nc.tensor.matmul(dcol, relu1, wd_s, start=True, stop=True)
                    nc.vector.tensor_copy(dstrip[:, t_out:t_out + 1], dcol)

                # ---------- L3 tail stats (feed chain(s+1)) ----------
                if do_l3tail:
                    mv3 = get_mv(s + 1)
                    st3 = work.tile([H3, 6], f32, tag="st3")
                    nc.vector.bn_stats(st3, h3q[s - 1][0:H3, :])
                    nc.vector.bn_aggr(mv3[0:H3, 4, :], st3)

                # ---------- L3 gates + cell @ t=s-4 ----------
                if do_l3:
                    z2T_in = z2q[s - 2]
                    # L3 gate cols in gL3: ifo [0:96), g [96:128)
                    for dst, wsrc, width in [(L3_IFO, 0, 96), (L3_G, 96, 32)]:
                        oc = g3f[:, dst:dst + width]
                        wc = slice(wsrc, wsrc + width)
                        nc.tensor.matmul(oc, z2T_in[:, 0, :], w3t_s[:, 0, wc],
                                         start=True, stop=False,
                                         skip_group_check=True)
                        nc.tensor.matmul(oc, z2T_in[:, 1, :], w3t_s[:, 1, wc],
                                         start=False, stop=False,
                                         skip_group_check=True)
                        nc.tensor.matmul(oc, h3q[s - 1], wh3ta_s[:, wc],
                                         start=False, stop=True,
                                         skip_group_check=True)
                    sig_3 = work.tile([128, 96], bf16, tag="sg3")
                    nc.scalar.activation(sig_3, g3f[:, L3_IFO:L3_IFO + 96],
                                         AF.Sigmoid)
                    tg3 = work.tile([128, H3], bf16, tag="tgC")
                    nc.scalar.activation(tg3, g3f[:, L3_G:L3_G + H3],
                                         AF.Tanh)
                    cn3 = work.tile([128, H3], bf16, tag="cnC")
                    tm3 = work.tile([128, H3], bf16, tag="tmC")
                    nc.gpsimd.tensor_mul(cn3, sig_3[:, 32:64],
                                         c13[:, H1:H1 + H3])
                    nc.gpsimd.tensor_mul(tm3, sig_3[:, 0:32], tg3)
                    nc.gpsimd.tensor_add(c13[:, H1:H1 + H3], cn3, tm3)
                    sig_3_prev = sig_3

                if s <= R + 5:
                    # ---- BN coefficients from bn_aggr's (mean, var), all on
                    # DVE ([128,5] ops are ~65ns there and z-applies are DVE
                    # anyway): fast-inverse-sqrt with ONE Newton iteration,
                    # gamma/beta fold.  10 serial ops.
                    mv = get_mv(s)
                    t2 = work.tile([128, 5], f32, tag="t2")
                    u2 = work.tile([128, 5], f32, tag="u2")
                    y1 = work.tile([128, 5], f32, tag="y1")
                    ve = work.tile([128, 5], f32, tag="ve")
                    s_ = work.tile([128, 5], f32, tag="s_")
                    tt = work.tile([128, 5], f32, tag="tt")
                    nc.vector.tensor_scalar(
                        out=ve, in0=mv[:, :, 1], scalar1=EPS, scalar2=None,
                        op0=OP.add)
                    nc.vector.tensor_scalar(
                        out=t2.bitcast(u32), in0=ve.bitcast(u32),
                        scalar1=1, scalar2=None, op0=OP.arith_shift_right)
                    nc.vector.tensor_sub(y1.bitcast(u32), magic_t,
                                         t2.bitcast(u32))
                    nc.vector.tensor_mul(u2, y1, y1)
                    nc.vector.tensor_mul(t2, ve, u2)
                    nc.vector.tensor_scalar(out=u2, in0=t2,
                                            scalar1=-0.5, scalar2=1.5,
                                            op0=OP.mult, op1=OP.add)
                    nc.vector.tensor_mul(y1, y1, u2)
                    nc.vector.tensor_mul(s_, y1, gball_s[:, 0:5])
                    nc.vector.tensor_mul(u2, mv[:, :, 0], s_)
                    nc.vector.tensor_sub(tt, gball_s[:, 5:10], u2)

                # ---------- L2 sigmoid/tanh + cell update ----------
                if do_l2:
                    # split by PSUM bank: cell math waits only the i,f bank
                    sig2 = work.tile([128, 3 * H2], bf16, tag="sigB")
                    # sigma_o first: it gates next superstep's h2 tail, which
                    # is on the binding L2 loop
                    nc.scalar.activation(sig2[:, 512:768], g2[:, 512:768],
                                         AF.Sigmoid)
                    nc.scalar.activation(sig2[:, 0:512], g2[:, 0:512],
                                         AF.Sigmoid)
                    tg2 = work.tile([128, H2], bf16, tag="tgB")
                    nc.scalar.activation(tg2, g2[:, 768:1024], AF.Tanh)
                    cn2 = work.tile([128, H2], bf16, tag="cnB")
                    tm2 = work.tile([128, H2], bf16, tag="tmB")
                    nc.vector.tensor_mul(cn2, sig2[:, 256:512], c2)
                    nc.vector.tensor_mul(tm2, sig2[:, 0:256], tg2)
                    nc.vector.tensor_add(c2, cn2, tm2)
                    sig2_prev = sig2

                # ---- BN applies on the h-states saved last superstep ----
                if s < R:
                    z1T = ztp.tile([128, 2, 128], bf16, tag="z1")
                    for j in range(2):
                        nc.vector.tensor_scalar(
                            out=z1T[:, j, :], in0=h1T[:, j, :],
                            scalar1=s_[:, j:j + 1], scalar2=tt[:, j:j + 1],
                            op0=OP.mult, op1=OP.add)
                    z1T_prev = z1T
                if do_z2:
                    # chain(s-1) applied to h2T written by this superstep's
                    # tail == baseline's z2T(s-1)
                    z2T = ztp.tile([128, 2, 128], bf16, tag="z2")
                    for j in range(2):
                        nc.gpsimd.tensor_scalar(
                            out=z2T[:, j, :], in0=h2T[:, j, :],
                            scalar1=s_prev[:, 2 + j:3 + j],
                            scalar2=tt_prev[:, 2 + j:3 + j],
                            op0=OP.mult, op1=OP.add)
                    z2q[s - 1] = z2T
                if 6 <= s <= R + 5:
                    z3T = ztp.tile([H3, 128], bf16, tag="z3")
                    nc.gpsimd.tensor_scalar(
                        out=z3T, in0=h3q[s - 2][0:H3, :],
                        scalar1=s_[0:H3, 4:5], scalar2=tt[0:H3, 4:5],
                        op0=OP.mult, op1=OP.add)
                    z3T_prev = z3T
                if s <= R + 5:
                    s_prev, tt_prev = s_, tt
                for k in [k for k in h3q if k <= s - 2]:
                    del h3q[k]
                for k in [k for k in z2q if k <= s - 2]:
                    del z2q[k]

            # ---------- amortized head sigmoid over the whole strip ----------
            # out_sb[:, 0, :] = sigmoid(d + c), out_sb[:, 1, :] = 1 - that;
            # the DMA interleaves them into y's (t, class) column order.
            nc.scalar.activation(out_sb[:, 0, :], dstrip, AF.Sigmoid,
                                 bias=headc_s, scale=1.0)
            nc.vector.tensor_scalar(
                out=out_sb[:, 1, :], in0=out_sb[:, 0, :],
                scalar1=-1.0, scalar2=1.0, op0=OP.mult, op1=OP.add)

            y_tc = y.rearrange("b (t two) -> b t two", two=2)
            nc.sync.dma_start(y_tc[:, :, 0], out_sb[:, 0, :])
            nc.sync.dma_start(y_tc[:, :, 1], out_sb[:, 1, :])

    nc.compile()
    return nc


def _prep_host(inputs, np_w, np_x):
    gp1 = _gate_perm(H1)
    gp2 = _gate_perm(H2)
    gp3 = _gate_perm(H3)
    f = lambda a: np.ascontiguousarray(a, dtype=np.float32)

    import ml_dtypes
    bf = ml_dtypes.bfloat16
    w1t = f(inputs["Wih1"][gp1].T).astype(np_w)
    wh1t = f(inputs["Whh1"][gp1].T).astype(bf)
    w2t = f(inputs["Wih2"][gp2].T).astype(bf)
    wh2t = f(inputs["Whh2"][gp2].T).astype(bf)
    w3t = f(inputs["Wih3"][gp3].T).astype(bf)
    wh3t = f(inputs["Whh3"][gp3].T).astype(bf)
    b1 = f(inputs["bih1"] + inputs["bhh1"])[gp1][None, :]
    b2 = f(inputs["bih2"] + inputs["bhh2"])[gp2][None, :]
    b3 = f(inputs["bih3"] + inputs["bhh3"])[gp3][None, :]
    wh3ta = np.concatenate([wh3t, b3.astype(bf)], axis=0)

    def cols128(v):  # [256] -> [128, 2]
        return np.ascontiguousarray(v.reshape(2, 128).T, dtype=np.float32)

    gball = np.zeros((128, 10), np.float32)
    gball[:, 0:2] = cols128(f(inputs["g1"]))
    gball[:, 2:4] = cols128(f(inputs["g2"]))
    gball[0:H3, 4] = f(inputs["g3"])
    gball[:, 5:7] = cols128(f(inputs["b1"]))
    gball[:, 7:9] = cols128(f(inputs["b2"]))
    gball[0:H3, 9] = f(inputs["b3"])

    wlt = f(inputs["Wl"].T).astype(bf)
    blp = f(inputs["bl"])[:, None]
    wd = f(inputs["Wl2"][0] - inputs["Wl2"][1])[:, None].astype(bf)
    dc = float(inputs["bl2"][0] - inputs["bl2"][1])
    headc = np.full((128, 1), dc, np.float32)

    shared = dict(w1t=w1t, wh1t=wh1t, w2t=w2t, wh2t=wh2t, w3t=w3t,
                  wh3ta=wh3ta, b1r=b1, b2r=b2, gball=gball,
                  wlt=wlt, blp=blp, wd=wd, headc=headc)

    x = np.asarray(inputs["x"], dtype=np.float32)
    in_maps = []
    for c in range(NCORES):
        xc = x[c * BL:(c + 1) * BL]
        xTc = np.ascontiguousarray(
            xc[:, :T_STEPS, :].transpose(2, 1, 0)).astype(np_x)
        m = dict(shared)
        m["xT"] = xTc
        in_maps.append(m)
    return in_maps


def kernel(**inputs):
    import concourse.mybir as mybir
    from concourse import bass_utils

    dt_w = mybir.dt.float32r
    dt_x = mybir.dt.float32r
    np_w = np.float32
    np_x = np.float32

    key = ("v4", str(dt_w), str(dt_x), T_STEPS, T_RUN)
    if key not in _CACHE:
        _CACHE[key] = _build(dt_w, dt_x, run=T_RUN)
    nc = _CACHE[key]

    in_maps = _prep_host(inputs, np_w, np_x)
    res = bass_utils.run_bass_kernel_spmd(nc, in_maps,
                                          core_ids=list(range(NCORES)))
    out = np.empty((B, T_STEPS, 2), np.float32)
    for c in range(NCORES):
        out[c * BL:(c + 1) * BL] = res.results[c]["y"].reshape(BL, T_STEPS, 2)
    return out

